# revision 18
# baseline (speedup 1.0000x reference)
# Trainium2 Bass kernel for DiffJPEG (nn_DiffJPEG_40587440947390).
#
# Pipeline per image: RGB->YCbCr + 4:2:0 subsample + per-8x8-block DCT +
# quantize + differentiable round + dequantize + IDCT + upsample + YCbCr->RGB
# + clip. Everything linear is folded into PE matmul stationaries; the only
# elementwise work is quantize/round/cube/dequantize and the final clip.
# All +-128 shifts cancel exactly (integer shifts commute with round).
#
# Sharding: pure data parallel, 2 images per core across 8 cores.
import os
from contextlib import ExitStack

import numpy as np

import bass_rust
import concourse.bass as bass
import concourse.tile as tile
from concourse import mybir
from concourse.bass_utils import run_bass_kernel_spmd
from concourse.masks import make_identity

F32 = mybir.dt.float32
N_CORES = 8
B_PER_CORE = 2

# fwd matmuls feed diff_round (precision-sensitive); inverse side does not.
FWD_F32R = os.environ.get("K_FWD_F32R", "0") == "1"
T2_F32R = os.environ.get("K_T2_F32R", "1") == "1"

# ---------------------------------------------------------------------------
# constants (mirror reference.py exactly)
# ---------------------------------------------------------------------------
QUALITY = 80
FACTOR = (200.0 - 2.0 * QUALITY) / 100.0

Y_TABLE = np.array([
    [16, 11, 10, 16, 24, 40, 51, 61],
    [12, 12, 14, 19, 26, 58, 60, 55],
    [14, 13, 16, 24, 40, 57, 69, 56],
    [14, 17, 22, 29, 51, 87, 80, 62],
    [18, 22, 37, 56, 68, 109, 103, 77],
    [24, 35, 55, 64, 81, 104, 113, 92],
    [49, 64, 78, 87, 103, 121, 120, 101],
    [72, 92, 95, 98, 112, 100, 103, 99]], dtype=np.float32).T

C_TABLE = np.full((8, 8), 99.0, dtype=np.float32)
C_TABLE[:4, :4] = np.array([
    [17, 18, 24, 47],
    [18, 21, 26, 66],
    [24, 26, 56, 99],
    [47, 66, 99, 99]], dtype=np.float32).T

_k = np.arange(8, dtype=np.float32)
_COS = np.cos((2.0 * _k[:, None] + 1.0) * _k[None, :] * np.pi / 16.0).astype(np.float32)
_ALPHA = np.array([1.0 / np.sqrt(2.0)] + [1.0] * 7, dtype=np.float32)

RGB2YCC = np.array([[0.299, 0.587, 0.114],
                    [-0.168736, -0.331264, 0.5],
                    [0.5, -0.418688, -0.081312]], dtype=np.float32)
YCC2RGB = np.array([[1.0, 0.0, 1.402],
                    [1.0, -0.344136, -0.714136],
                    [1.0, 1.772, 0.0]], dtype=np.float32)

MAGIC = float(np.float32(1.5 * 2.0 ** 23))

G8 = (_COS * (_ALPHA[None, :] * 0.5)).astype(np.float32)     # [x, u]
H8 = (_COS.T * (_ALPHA[:, None] * 0.5)).astype(np.float32)   # [freq, pixel]


def _blockdiag(M, n):
    b0, b1 = M.shape
    out = np.zeros((b0 * n, b1 * n), dtype=np.float32)
    for i in range(n):
        out[i * b0:(i + 1) * b0, i * b1:(i + 1) * b1] = M
    return out


def _pool_mat(nrows):
    P = np.zeros((nrows, nrows // 2), dtype=np.float32)
    for h in range(nrows):
        P[h, h // 2] = 1.0
    return P


def _up_mat(nsmall):
    U = np.zeros((nsmall, 2 * nsmall), dtype=np.float32)
    for w in range(2 * nsmall):
        U[w // 2, w] = 1.0
    return U


def build_mats():
    m = {}
    BD16G = _blockdiag(G8, 16)
    BD16H = _blockdiag(H8, 16)
    BD8G = _blockdiag(G8, 8)
    BD8H = _blockdiag(H8, 8)
    P128 = _pool_mat(128)
    U64 = _up_mat(64)

    m['S1Y'] = [(np.float32(255.0 * RGB2YCC[0, c]) * BD16G).astype(np.float32)
                for c in range(3)]
    E = (P128 @ BD8G).astype(np.float32)
    m['S1C'] = [np.concatenate([
        np.float32(255.0 * 0.25 * RGB2YCC[1, c]) * E,
        np.float32(255.0 * 0.25 * RGB2YCC[2, c]) * E], axis=1).astype(np.float32)
        for c in range(3)]
    m['S2'] = BD16G
    m['S3'] = BD16H
    s3c = []
    for q in range(2):
        M2 = np.zeros((128, 128), dtype=np.float32)
        M2[64 * q:64 * q + 64, :] = BD8H @ U64
        s3c.append(M2.astype(np.float32))
    m['S3C'] = s3c
    m['S4Y'] = [(np.float32(YCC2RGB[c, 0] / 255.0) * BD16H).astype(np.float32)
                for c in range(3)]
    m['S4C'] = [np.concatenate([
        np.float32(YCC2RGB[c, 1] / 255.0) * (BD8H @ U64),
        np.float32(YCC2RGB[c, 2] / 255.0) * (BD8H @ U64)], axis=0).astype(np.float32)
        for c in range(3)]

    p = np.arange(128)[:, None]
    f = np.arange(512)[None, :]
    m['RY'] = (1.0 / (Y_TABLE[f % 8, p % 8] * FACTOR)).astype(np.float32)
    m['RYinv'] = (Y_TABLE[f % 8, p % 8] * FACTOR).astype(np.float32)
    m['RC'] = (1.0 / (C_TABLE[f % 8, p % 8] * FACTOR)).astype(np.float32)
    m['RCinv'] = (C_TABLE[f % 8, p % 8] * FACTOR).astype(np.float32)
    return m


def split_multi_waits(nc):
    """The walrus build in this container accepts at most one sync wait per
    instruction; Tile attaches several. Split the extras onto standalone
    NoOp carriers immediately before each instruction (same engine)."""
    cnt = 0
    for f in nc.m.functions:
        for bb in f.blocks:
            old = list(bb.instructions)
            new = []
            changed = False
            for inst in old:
                si = inst.sync_info
                if si is not None and len(si.on_wait) > 1:
                    waits = list(si.on_wait)
                    for w in waits[:-1]:
                        cnt += 1
                        nop = bass_rust.InstNoOp(name=f"wnop-{cnt}", ins=[],
                                                 outs=[])
                        nop.engine = inst.engine
                        nop.sync_info = mybir.SyncInfo(on_wait=[w],
                                                       on_update=[])
                        new.append(nop)
                    inst.sync_info = mybir.SyncInfo(
                        on_wait=[waits[-1]], on_update=list(si.on_update))
                    changed = True
                new.append(inst)
            if changed:
                bb.instructions = new
    return cnt


# ---------------------------------------------------------------------------
# kernel builder
# ---------------------------------------------------------------------------
def build(nc: bass.Bass):
    AL = mybir.AluOpType
    m = build_mats()

    x = nc.dram_tensor("x", [B_PER_CORE, 3, 512, 512], F32,
                       kind="ExternalInput").ap()
    out = nc.dram_tensor("out", [B_PER_CORE, 3, 512, 512], F32,
                         kind="ExternalOutput").ap()

    # constants baked into the NEFF
    s1y_d = nc.inline_tensor(np.stack(m['S1Y']), "c_s1y").ap()
    s1c_d = nc.inline_tensor(np.stack(m['S1C']), "c_s1c").ap()
    s2_d = nc.inline_tensor(m['S2'], "c_s2").ap()
    s3_d = nc.inline_tensor(m['S3'], "c_s3").ap()
    s3c_d = nc.inline_tensor(np.stack(m['S3C']), "c_s3c").ap()
    s4y_d = nc.inline_tensor(np.stack(m['S4Y']), "c_s4y").ap()
    s4c_d = nc.inline_tensor(np.stack(m['S4C']), "c_s4c").ap()
    r_d = nc.inline_tensor(np.stack([m['RY'], m['RYinv'], m['RC'], m['RCinv']]),
                           "c_r").ap()

    with tile.TileContext(nc) as tc, ExitStack() as ctx:
        cpool = ctx.enter_context(tc.tile_pool(name="consts", bufs=1))

        def cload(dram_ap, shape, tag):
            t = cpool.tile(shape, F32, tag=tag)
            nc.sync.dma_start(t[:], dram_ap)
            return t

        s1y = [cload(s1y_d[c], [128, 128], f"s1y{c}") for c in range(3)]
        s1c = [cload(s1c_d[c], [128, 128], f"s1c{c}") for c in range(3)]
        s2 = cload(s2_d, [128, 128], "s2")
        s3 = cload(s3_d, [128, 128], "s3")
        s3c = [cload(s3c_d[q], [128, 128], f"s3c{q}") for q in range(2)]
        s4y = [cload(s4y_d[c], [128, 128], f"s4y{c}") for c in range(3)]
        s4c = [cload(s4c_d[c], [128, 128], f"s4c{c}") for c in range(3)]
        rt = [cload(r_d[j], [128, 512], f"r{j}") for j in range(4)]
        RY, RYI, RC, RCI = rt

        ident = cpool.tile([128, 128], F32, tag="ident")
        make_identity(nc, ident[:])

        # float32r copies of inverse-side constants (PE requires fp32r
        # matmul inputs to be produced as rounded float32r)
        R32 = mybir.dt.float32r

        def to_r32(src_t, tag):
            t = cpool.tile([128, 128], R32, tag=tag)
            nc.scalar.copy(t[:], src_t[:])
            return t

        s3r = to_r32(s3, "s3r")
        s3cr = [to_r32(s3c[q], f"s3cr{q}") for q in range(2)]
        s4yr = [to_r32(s4y[c], f"s4yr{c}") for c in range(3)]
        s4cr = [to_r32(s4c[c], f"s4cr{c}") for c in range(3)]
        identr = to_r32(ident, "identr")

        # SBUF pools
        xp = ctx.enter_context(tc.tile_pool(name="x", bufs=14))
        wp = ctx.enter_context(tc.tile_pool(name="wpool", bufs=8))
        p1e = ctx.enter_context(tc.tile_pool(name="p1evac", bufs=6))
        t1e = ctx.enter_context(tc.tile_pool(name="t1evac", bufs=7))
        vch = ctx.enter_context(tc.tile_pool(name="vchain", bufs=3))
        yqp = ctx.enter_context(tc.tile_pool(name="yq", bufs=13))
        p3e = ctx.enter_context(tc.tile_pool(name="p3evac", bufs=6))
        t2e = ctx.enter_context(tc.tile_pool(name="t2evac", bufs=10))
        osb = ctx.enter_context(tc.tile_pool(name="outsb", bufs=5))

        # PSUM pools (8 banks total)
        p1p = ctx.enter_context(tc.tile_pool(name="p1p", bufs=2, space="PSUM"))
        p2p = ctx.enter_context(tc.tile_pool(name="p2p", bufs=1, space="PSUM"))
        tpp = ctx.enter_context(tc.tile_pool(name="tpp", bufs=1, space="PSUM"))
        p3p = ctx.enter_context(tc.tile_pool(name="p3p", bufs=1, space="PSUM"))
        p4p = ctx.enter_context(tc.tile_pool(name="p4p", bufs=2, space="PSUM"))

        def fcast(ap):
            return ap.bitcast(R32) if FWD_F32R else ap

        BF16 = mybir.dt.bfloat16

        def fwd(b):
            # ---- load x tiles ----
            xt = [[None] * 4 for _ in range(3)]
            for c in range(3):
                for i in range(4):
                    t = xp.tile([128, 512], F32, tag="x")
                    nc.sync.dma_start(t[:], x[b, c, 128 * i:128 * (i + 1), :])
                    xt[c][i] = t

            # ---- horizontal (w) pool for chroma path (gpsimd) ----
            pw = [[None] * 4 for _ in range(3)]
            for c in range(3):
                for i in range(4):
                    t = wp.tile([128, 256], F32, tag="pw")
                    xr = xt[c][i][:].rearrange("p (w t) -> p w t", t=2)
                    nc.gpsimd.tensor_tensor(
                        out=t[:], in0=xr[:, :, 0], in1=xr[:, :, 1], op=AL.add)
                    pw[c][i] = t

            # ---- pass-1 Y: [u, w] ----
            yt1 = []
            for i in range(4):
                ps = p1p.tile([128, 512], F32, tag="p1")
                for c in range(3):
                    nc.tensor.matmul(ps[:], lhsT=fcast(s1y[c][:]),
                                     rhs=fcast(xt[c][i][:]),
                                     start=(c == 0), stop=(c == 2))
                ev = p1e.tile([128, 512], F32, tag="yt1")
                nc.scalar.copy(ev[:], ps[:])
                yt1.append(ev)

            # ---- pass-1 chroma: [cb_u2|cr_u2, w2] ----
            ct1 = []
            for i in range(4):
                ps = p1p.tile([128, 256], F32, tag="p1")
                for c in range(3):
                    nc.tensor.matmul(ps[:], lhsT=fcast(s1c[c][:]),
                                     rhs=fcast(pw[c][i][:]),
                                     start=(c == 0), stop=(c == 2))
                ev = p1e.tile([128, 256], F32, tag="ct1")
                nc.scalar.copy(ev[:], ps[:])
                ct1.append(ev)

            # ---- T1 (PE transposes) ----
            yt1T = []
            for t_ in range(4):
                ps = tpp.tile([128, 512], F32, tag="tp")
                for i in range(4):
                    nc.tensor.transpose(
                        ps[:, 128 * i:128 * (i + 1)],
                        yt1[i][:, 128 * t_:128 * (t_ + 1)], ident[:])
                ev = t1e.tile([128, 512], F32, tag="t1ev")
                nc.vector.tensor_copy(out=ev[:], in_=ps[:])
                yt1T.append(ev)
            ct1T = []
            for k in range(2):
                ps = tpp.tile([128, 512], F32, tag="tp")
                for i in range(4):
                    nc.tensor.transpose(
                        ps[:, 128 * i:128 * (i + 1)],
                        ct1[i][:, 128 * k:128 * (k + 1)], ident[:])
                ev = t1e.tile([128, 512], F32, tag="t1ev")
                nc.scalar.copy(ev[:], ps[:])
                ct1T.append(ev)

            # ---- pass-2 + quant + diff_round + dequant ----
            def round_chain(zps, R, Rinv):
                v = vch.tile([128, 512], F32, tag="v")
                nc.vector.tensor_tensor(out=v[:], in0=zps[:], in1=R[:],
                                        op=AL.mult)
                r = vch.tile([128, 512], F32, tag="r")
                nc.vector.tensor_scalar(out=r[:], in0=v[:], scalar1=MAGIC,
                                        scalar2=MAGIC, op0=AL.add,
                                        op1=AL.subtract)
                d = vch.tile([128, 512], BF16, tag="d")
                nc.gpsimd.tensor_tensor(out=d[:], in0=v[:], in1=r[:],
                                        op=AL.subtract)
                p2 = vch.tile([128, 512], BF16, tag="p2")
                nc.scalar.square(p2[:], d[:])
                c3 = vch.tile([128, 512], BF16, tag="c3")
                nc.vector.tensor_tensor(out=c3[:], in0=d[:], in1=p2[:],
                                        op=AL.mult)
                yq_q = vch.tile([128, 512], F32, tag="yqq")
                nc.vector.tensor_tensor(out=yq_q[:], in0=r[:], in1=c3[:],
                                        op=AL.add)
                yq = yqp.tile([128, 512], R32, tag="yq")
                nc.gpsimd.tensor_tensor(out=yq[:], in0=yq_q[:], in1=Rinv[:],
                                        op=AL.mult)
                return yq

            YQ = []
            for t_ in range(4):
                ps = p2p.tile([128, 512], F32, tag="p2")
                nc.tensor.matmul(ps[:], lhsT=fcast(s2[:]),
                                 rhs=fcast(yt1T[t_][:]),
                                 start=True, stop=True)
                YQ.append(round_chain(ps, RY, RYI))
            CQ = []
            for k in range(2):
                ps = p2p.tile([128, 512], F32, tag="p2")
                nc.tensor.matmul(ps[:], lhsT=fcast(s2[:]),
                                 rhs=fcast(ct1T[k][:]),
                                 start=True, stop=True)
                CQ.append(round_chain(ps, RC, RCI))
            return YQ, CQ

        def inv(b, YQ, CQ):
            # ---- pass-3 (float32r) ----
            py = []
            for t_ in range(4):
                ps = p3p.tile([128, 512], F32, tag="p3")
                nc.tensor.matmul(ps[:], lhsT=s3r[:], rhs=YQ[t_][:],
                                 start=True, stop=True)
                ev = p3e.tile([128, 512], R32, tag="p3ev")
                nc.scalar.copy(ev[:], ps[:])
                py.append(ev)
            pc = []
            for t_ in range(4):
                ps = p3p.tile([128, 512], F32, tag="p3")
                nc.tensor.matmul(ps[:], lhsT=s3cr[t_ % 2][:],
                                 rhs=CQ[t_ // 2][:],
                                 start=True, stop=True)
                ev = p3e.tile([128, 512], R32, tag="p3ev")
                nc.scalar.copy(ev[:], ps[:])
                pc.append(ev)

            # ---- T2 (fp32r transposes) ----
            yu, cu = [], []
            for i in range(4):
                ps = tpp.tile([128, 512], R32, tag="tpr")
                for t_ in range(4):
                    nc.tensor.transpose(
                        ps[:, 128 * t_:128 * (t_ + 1)],
                        py[t_][:, 128 * i:128 * (i + 1)], identr[:])
                ev = t2e.tile([128, 512], R32, tag="t2ev")
                nc.vector.tensor_copy(out=ev[:], in_=ps[:])
                yu.append(ev)
            for i in range(4):
                ps = tpp.tile([128, 512], R32, tag="tpr")
                for t_ in range(4):
                    nc.tensor.transpose(
                        ps[:, 128 * t_:128 * (t_ + 1)],
                        pc[t_][:, 128 * i:128 * (i + 1)], identr[:])
                ev = t2e.tile([128, 512], R32, tag="t2ev")
                nc.scalar.copy(ev[:], ps[:])
                cu.append(ev)

            # ---- pass-4 (+color) + clip + store ----
            for i in range(4):
                for c in range(3):
                    ps = p4p.tile([128, 512], F32, tag="p4")
                    nc.tensor.matmul(ps[:], lhsT=s4yr[c][:], rhs=yu[i][:],
                                     start=True, stop=False)
                    nc.tensor.matmul(ps[:], lhsT=s4cr[c][:], rhs=cu[i][:],
                                     start=False, stop=True)
                    ot = osb.tile([128, 512], F32, tag="ot")
                    if (i + c) % 3 == 2:
                        # spread some clips off DVE: ACT relu + gpsimd min
                        nc.scalar.activation(
                            ot[:], ps[:], mybir.ActivationFunctionType.Relu)
                        nc.gpsimd.tensor_scalar_min(out=ot[:], in0=ot[:],
                                                    scalar1=1.0)
                    else:
                        nc.vector.tensor_scalar(out=ot[:], in0=ps[:],
                                                scalar1=1.0, scalar2=0.0,
                                                op0=AL.min, op1=AL.max)
                    nc.sync.dma_start(out[b, c, 128 * i:128 * (i + 1), :],
                                      ot[:])

        # software pipeline: fwd(0), fwd(1) overlap inv(0), then inv(1)
        q0 = fwd(0)
        q1 = fwd(1)
        inv(0, *q0)
        inv(1, *q1)

    split_multi_waits(nc)
    return nc


# ---------------------------------------------------------------------------
# host entry point
# ---------------------------------------------------------------------------
_CACHE = {}


def _get_nc():
    if "nc" not in _CACHE:
        nc = bass.Bass("TRN2", target_bir_lowering=False, debug=False,
                       num_devices=N_CORES)
        build(nc)
        _CACHE["nc"] = nc
    return _CACHE["nc"]


def kernel(x: np.ndarray, source_image: np.ndarray, trace: bool = False,
           **run_kwargs):
    x = np.ascontiguousarray(np.asarray(x, dtype=np.float32))
    assert x.shape == (16, 3, 512, 512)
    nc = _get_nc()
    in_maps = [{"x": x[core * B_PER_CORE:(core + 1) * B_PER_CORE]}
               for core in range(N_CORES)]
    res = run_bass_kernel_spmd(nc, in_maps, core_ids=list(range(N_CORES)),
                               trace=trace, **run_kwargs)
    recovered = np.concatenate([res.results[c]["out"] for c in range(N_CORES)],
                               axis=0)
    _CACHE["last_results"] = res
    return recovered, source_image


# revision 20
# speedup vs baseline: 1.2130x; 1.2130x over previous
# Trainium2 Bass kernel for DiffJPEG (nn_DiffJPEG_40587440947390).
#
# Pipeline per image: RGB->YCbCr + 4:2:0 subsample + per-8x8-block DCT +
# quantize + differentiable round + dequantize + IDCT + upsample + YCbCr->RGB
# + clip. Everything linear is folded into PE matmul stationaries; the only
# elementwise work is quantize/round/cube/dequantize and the final clip.
# All +-128 shifts cancel exactly (integer shifts commute with round).
#
# Sharding: pure data parallel, 2 images per core across 8 cores.
import os
from contextlib import ExitStack

import numpy as np

import bass_rust
import concourse.bass as bass
import concourse.tile as tile
from concourse import mybir
from concourse.bass_utils import run_bass_kernel_spmd
from concourse.masks import make_identity

F32 = mybir.dt.float32
N_CORES = 8
B_PER_CORE = 2

# fwd matmuls feed diff_round (precision-sensitive); inverse side does not.
FWD_F32R = os.environ.get("K_FWD_F32R", "0") == "1"
T2_F32R = os.environ.get("K_T2_F32R", "1") == "1"

# ---------------------------------------------------------------------------
# constants (mirror reference.py exactly)
# ---------------------------------------------------------------------------
QUALITY = 80
FACTOR = (200.0 - 2.0 * QUALITY) / 100.0

Y_TABLE = np.array([
    [16, 11, 10, 16, 24, 40, 51, 61],
    [12, 12, 14, 19, 26, 58, 60, 55],
    [14, 13, 16, 24, 40, 57, 69, 56],
    [14, 17, 22, 29, 51, 87, 80, 62],
    [18, 22, 37, 56, 68, 109, 103, 77],
    [24, 35, 55, 64, 81, 104, 113, 92],
    [49, 64, 78, 87, 103, 121, 120, 101],
    [72, 92, 95, 98, 112, 100, 103, 99]], dtype=np.float32).T

C_TABLE = np.full((8, 8), 99.0, dtype=np.float32)
C_TABLE[:4, :4] = np.array([
    [17, 18, 24, 47],
    [18, 21, 26, 66],
    [24, 26, 56, 99],
    [47, 66, 99, 99]], dtype=np.float32).T

_k = np.arange(8, dtype=np.float32)
_COS = np.cos((2.0 * _k[:, None] + 1.0) * _k[None, :] * np.pi / 16.0).astype(np.float32)
_ALPHA = np.array([1.0 / np.sqrt(2.0)] + [1.0] * 7, dtype=np.float32)

RGB2YCC = np.array([[0.299, 0.587, 0.114],
                    [-0.168736, -0.331264, 0.5],
                    [0.5, -0.418688, -0.081312]], dtype=np.float32)
YCC2RGB = np.array([[1.0, 0.0, 1.402],
                    [1.0, -0.344136, -0.714136],
                    [1.0, 1.772, 0.0]], dtype=np.float32)

MAGIC = float(np.float32(1.5 * 2.0 ** 23))

G8 = (_COS * (_ALPHA[None, :] * 0.5)).astype(np.float32)     # [x, u]
H8 = (_COS.T * (_ALPHA[:, None] * 0.5)).astype(np.float32)   # [freq, pixel]


def _blockdiag(M, n):
    b0, b1 = M.shape
    out = np.zeros((b0 * n, b1 * n), dtype=np.float32)
    for i in range(n):
        out[i * b0:(i + 1) * b0, i * b1:(i + 1) * b1] = M
    return out


def _pool_mat(nrows):
    P = np.zeros((nrows, nrows // 2), dtype=np.float32)
    for h in range(nrows):
        P[h, h // 2] = 1.0
    return P


def _up_mat(nsmall):
    U = np.zeros((nsmall, 2 * nsmall), dtype=np.float32)
    for w in range(2 * nsmall):
        U[w // 2, w] = 1.0
    return U


def build_mats():
    m = {}
    BD16G = _blockdiag(G8, 16)
    BD16H = _blockdiag(H8, 16)
    BD8G = _blockdiag(G8, 8)
    BD8H = _blockdiag(H8, 8)
    P128 = _pool_mat(128)
    U64 = _up_mat(64)

    m['S1Y'] = [(np.float32(255.0 * RGB2YCC[0, c]) * BD16G).astype(np.float32)
                for c in range(3)]
    E = (P128 @ BD8G).astype(np.float32)
    m['S1C'] = [np.concatenate([
        np.float32(255.0 * 0.25 * RGB2YCC[1, c]) * E,
        np.float32(255.0 * 0.25 * RGB2YCC[2, c]) * E], axis=1).astype(np.float32)
        for c in range(3)]
    m['S2'] = BD16G
    m['S3'] = BD16H
    s3c = []
    for q in range(2):
        M2 = np.zeros((128, 128), dtype=np.float32)
        M2[64 * q:64 * q + 64, :] = BD8H @ U64
        s3c.append(M2.astype(np.float32))
    m['S3C'] = s3c
    m['S4Y'] = [(np.float32(YCC2RGB[c, 0] / 255.0) * BD16H).astype(np.float32)
                for c in range(3)]
    m['S4C'] = [np.concatenate([
        np.float32(YCC2RGB[c, 1] / 255.0) * (BD8H @ U64),
        np.float32(YCC2RGB[c, 2] / 255.0) * (BD8H @ U64)], axis=0).astype(np.float32)
        for c in range(3)]

    p = np.arange(128)[:, None]
    f = np.arange(512)[None, :]
    m['RY'] = (1.0 / (Y_TABLE[f % 8, p % 8] * FACTOR)).astype(np.float32)
    m['RYinv'] = (Y_TABLE[f % 8, p % 8] * FACTOR).astype(np.float32)
    m['RC'] = (1.0 / (C_TABLE[f % 8, p % 8] * FACTOR)).astype(np.float32)
    m['RCinv'] = (C_TABLE[f % 8, p % 8] * FACTOR).astype(np.float32)
    return m


def split_multi_waits(nc):
    """The walrus build in this container accepts at most one sync wait per
    instruction; Tile attaches several. Split the extras onto standalone
    NoOp carriers immediately before each instruction (same engine)."""
    cnt = 0
    for f in nc.m.functions:
        for bb in f.blocks:
            old = list(bb.instructions)
            new = []
            changed = False
            for inst in old:
                si = inst.sync_info
                if si is not None and len(si.on_wait) > 1:
                    waits = list(si.on_wait)
                    for w in waits[:-1]:
                        cnt += 1
                        nop = bass_rust.InstNoOp(name=f"wnop-{cnt}", ins=[],
                                                 outs=[])
                        nop.engine = inst.engine
                        nop.sync_info = mybir.SyncInfo(on_wait=[w],
                                                       on_update=[])
                        new.append(nop)
                    inst.sync_info = mybir.SyncInfo(
                        on_wait=[waits[-1]], on_update=list(si.on_update))
                    changed = True
                new.append(inst)
            if changed:
                bb.instructions = new
    return cnt


# ---------------------------------------------------------------------------
# kernel builder
# ---------------------------------------------------------------------------
def build(nc: bass.Bass):
    AL = mybir.AluOpType
    m = build_mats()

    x = nc.dram_tensor("x", [B_PER_CORE, 3, 512, 512], F32,
                       kind="ExternalInput").ap()
    out = nc.dram_tensor("out", [B_PER_CORE, 3, 512, 512], F32,
                         kind="ExternalOutput").ap()

    # constants baked into the NEFF
    s1y_d = nc.inline_tensor(np.stack(m['S1Y']), "c_s1y").ap()
    s1c_d = nc.inline_tensor(np.stack(m['S1C']), "c_s1c").ap()
    s2_d = nc.inline_tensor(m['S2'], "c_s2").ap()
    s3_d = nc.inline_tensor(m['S3'], "c_s3").ap()
    s3c_d = nc.inline_tensor(np.stack(m['S3C']), "c_s3c").ap()
    s4y_d = nc.inline_tensor(np.stack(m['S4Y']), "c_s4y").ap()
    s4c_d = nc.inline_tensor(np.stack(m['S4C']), "c_s4c").ap()
    r_d = nc.inline_tensor(np.stack([m['RY'], m['RYinv'], m['RC'], m['RCinv']]),
                           "c_r").ap()

    with tile.TileContext(nc) as tc, ExitStack() as ctx:
        cpool = ctx.enter_context(tc.tile_pool(name="consts", bufs=1))

        def cload(dram_ap, shape, tag):
            t = cpool.tile(shape, F32, tag=tag)
            nc.sync.dma_start(t[:], dram_ap)
            return t

        s1y = [cload(s1y_d[c], [128, 128], f"s1y{c}") for c in range(3)]
        s1c = [cload(s1c_d[c], [128, 128], f"s1c{c}") for c in range(3)]
        s2 = cload(s2_d, [128, 128], "s2")
        s3 = cload(s3_d, [128, 128], "s3")
        s3c = [cload(s3c_d[q], [128, 128], f"s3c{q}") for q in range(2)]
        s4y = [cload(s4y_d[c], [128, 128], f"s4y{c}") for c in range(3)]
        s4c = [cload(s4c_d[c], [128, 128], f"s4c{c}") for c in range(3)]
        rt = [cload(r_d[j], [128, 512], f"r{j}") for j in range(4)]
        RY, RYI, RC, RCI = rt

        ident = cpool.tile([128, 128], F32, tag="ident")
        make_identity(nc, ident[:])

        # float32r copies of inverse-side constants (PE requires fp32r
        # matmul inputs to be produced as rounded float32r)
        R32 = mybir.dt.float32r

        def to_r32(src_t, tag):
            t = cpool.tile([128, 128], R32, tag=tag)
            nc.scalar.copy(t[:], src_t[:])
            return t

        s3r = to_r32(s3, "s3r")
        s3cr = [to_r32(s3c[q], f"s3cr{q}") for q in range(2)]
        s4yr = [to_r32(s4y[c], f"s4yr{c}") for c in range(3)]
        s4cr = [to_r32(s4c[c], f"s4cr{c}") for c in range(3)]
        identr = to_r32(ident, "identr")

        # SBUF pools
        xp = ctx.enter_context(tc.tile_pool(name="x", bufs=16))
        wp = ctx.enter_context(tc.tile_pool(name="wpool", bufs=8))
        p1e = ctx.enter_context(tc.tile_pool(name="p1evac", bufs=6))
        t1e = ctx.enter_context(tc.tile_pool(name="t1evac", bufs=7))
        vch = ctx.enter_context(tc.tile_pool(name="vchain", bufs=4))
        yqp = ctx.enter_context(tc.tile_pool(name="yq", bufs=8))
        p3e = ctx.enter_context(tc.tile_pool(name="p3evac", bufs=6))
        t2e = ctx.enter_context(tc.tile_pool(name="t2evac", bufs=10))
        osb = ctx.enter_context(tc.tile_pool(name="outsb", bufs=6))

        # PSUM pools (8 banks total)
        p1p = ctx.enter_context(tc.tile_pool(name="p1p", bufs=2, space="PSUM"))
        p2p = ctx.enter_context(tc.tile_pool(name="p2p", bufs=1, space="PSUM"))
        tpp = ctx.enter_context(tc.tile_pool(name="tpp", bufs=1, space="PSUM"))
        p3p = ctx.enter_context(tc.tile_pool(name="p3p", bufs=1, space="PSUM"))
        p4p = ctx.enter_context(tc.tile_pool(name="p4p", bufs=2, space="PSUM"))

        def fcast(ap):
            return ap.bitcast(R32) if FWD_F32R else ap

        for b in range(B_PER_CORE):
            # ---- load x tiles ----
            xt = [[None] * 4 for _ in range(3)]
            for c in range(3):
                for i in range(4):
                    t = xp.tile([128, 512], F32, tag="x")
                    nc.sync.dma_start(t[:], x[b, c, 128 * i:128 * (i + 1), :])
                    xt[c][i] = t

            # ---- horizontal (w) pool for chroma path (gpsimd) ----
            pw = [[None] * 4 for _ in range(3)]
            for c in range(3):
                for i in range(4):
                    t = wp.tile([128, 256], F32, tag="pw")
                    xr = xt[c][i][:].rearrange("p (w t) -> p w t", t=2)
                    nc.gpsimd.tensor_tensor(
                        out=t[:], in0=xr[:, :, 0], in1=xr[:, :, 1], op=AL.add)
                    pw[c][i] = t

            # ---- pass-1 Y: [u, w] ----
            yt1 = []
            for i in range(4):
                ps = p1p.tile([128, 512], F32, tag="p1")
                for c in range(3):
                    nc.tensor.matmul(ps[:], lhsT=fcast(s1y[c][:]),
                                     rhs=fcast(xt[c][i][:]),
                                     start=(c == 0), stop=(c == 2))
                ev = p1e.tile([128, 512], F32, tag="yt1")
                nc.scalar.copy(ev[:], ps[:])
                yt1.append(ev)

            # ---- pass-1 chroma: [cb_u2|cr_u2, w2] ----
            ct1 = []
            for i in range(4):
                ps = p1p.tile([128, 256], F32, tag="p1")
                for c in range(3):
                    nc.tensor.matmul(ps[:], lhsT=fcast(s1c[c][:]),
                                     rhs=fcast(pw[c][i][:]),
                                     start=(c == 0), stop=(c == 2))
                ev = p1e.tile([128, 256], F32, tag="ct1")
                nc.scalar.copy(ev[:], ps[:])
                ct1.append(ev)

            # ---- T1 (PE transposes) ----
            yt1T = []
            for t_ in range(4):
                ps = tpp.tile([128, 512], F32, tag="tp")
                for i in range(4):
                    nc.tensor.transpose(
                        ps[:, 128 * i:128 * (i + 1)],
                        yt1[i][:, 128 * t_:128 * (t_ + 1)], ident[:])
                ev = t1e.tile([128, 512], F32, tag="t1ev")
                nc.vector.tensor_copy(out=ev[:], in_=ps[:])
                yt1T.append(ev)
            ct1T = []
            for k in range(2):
                ps = tpp.tile([128, 512], F32, tag="tp")
                for i in range(4):
                    nc.tensor.transpose(
                        ps[:, 128 * i:128 * (i + 1)],
                        ct1[i][:, 128 * k:128 * (k + 1)], ident[:])
                ev = t1e.tile([128, 512], F32, tag="t1ev")
                nc.scalar.copy(ev[:], ps[:])
                ct1T.append(ev)

            # ---- pass-2 + quant + diff_round + dequant ----
            def round_chain(zps, R, Rinv):
                v = vch.tile([128, 512], F32, tag="v")
                nc.vector.tensor_tensor(out=v[:], in0=zps[:], in1=R[:], op=AL.mult)
                r = vch.tile([128, 512], F32, tag="r")
                nc.vector.tensor_scalar(out=r[:], in0=v[:], scalar1=MAGIC,
                                        scalar2=MAGIC, op0=AL.add, op1=AL.subtract)
                d = vch.tile([128, 512], F32, tag="d")
                nc.gpsimd.tensor_tensor(out=d[:], in0=v[:], in1=r[:], op=AL.subtract)
                p2 = vch.tile([128, 512], F32, tag="p2")
                nc.scalar.square(p2[:], d[:])
                c3 = vch.tile([128, 512], F32, tag="c3")
                nc.gpsimd.tensor_tensor(out=c3[:], in0=d[:], in1=p2[:], op=AL.mult)
                yq_q = vch.tile([128, 512], F32, tag="yqq")
                nc.vector.tensor_tensor(out=yq_q[:], in0=r[:], in1=c3[:], op=AL.add)
                yq = yqp.tile([128, 512], R32, tag="yq")
                nc.gpsimd.tensor_tensor(out=yq[:], in0=yq_q[:], in1=Rinv[:],
                                        op=AL.mult)
                return yq

            YQ = []
            for t_ in range(4):
                ps = p2p.tile([128, 512], F32, tag="p2")
                nc.tensor.matmul(ps[:], lhsT=fcast(s2[:]),
                                 rhs=fcast(yt1T[t_][:]),
                                 start=True, stop=True)
                YQ.append(round_chain(ps, RY, RYI))
            CQ = []
            for k in range(2):
                ps = p2p.tile([128, 512], F32, tag="p2")
                nc.tensor.matmul(ps[:], lhsT=fcast(s2[:]),
                                 rhs=fcast(ct1T[k][:]),
                                 start=True, stop=True)
                CQ.append(round_chain(ps, RC, RCI))

            # ---- pass-3 (float32r: post-round side tolerates reduced precision) ----
            py = []
            for t_ in range(4):
                ps = p3p.tile([128, 512], F32, tag="p3")
                nc.tensor.matmul(ps[:], lhsT=s3r[:], rhs=YQ[t_][:],
                                 start=True, stop=True)
                ev = p3e.tile([128, 512], R32, tag="p3ev")
                nc.scalar.copy(ev[:], ps[:])
                py.append(ev)
            pc = []
            for t_ in range(4):
                ps = p3p.tile([128, 512], F32, tag="p3")
                nc.tensor.matmul(ps[:], lhsT=s3cr[t_ % 2][:],
                                 rhs=CQ[t_ // 2][:],
                                 start=True, stop=True)
                ev = p3e.tile([128, 512], R32, tag="p3ev")
                nc.scalar.copy(ev[:], ps[:])
                pc.append(ev)

            # ---- T2 (fp32r transposes) ----
            yu, cu = [], []
            for i in range(4):
                ps = tpp.tile([128, 512], R32, tag="tpr")
                for t_ in range(4):
                    nc.tensor.transpose(
                        ps[:, 128 * t_:128 * (t_ + 1)],
                        py[t_][:, 128 * i:128 * (i + 1)], identr[:])
                ev = t2e.tile([128, 512], R32, tag="t2ev")
                nc.vector.tensor_copy(out=ev[:], in_=ps[:])
                yu.append(ev)
            for i in range(4):
                ps = tpp.tile([128, 512], R32, tag="tpr")
                for t_ in range(4):
                    nc.tensor.transpose(
                        ps[:, 128 * t_:128 * (t_ + 1)],
                        pc[t_][:, 128 * i:128 * (i + 1)], identr[:])
                ev = t2e.tile([128, 512], R32, tag="t2ev")
                nc.scalar.copy(ev[:], ps[:])
                cu.append(ev)

            # ---- pass-4 (+color) + clip + store ----
            for i in range(4):
                for c in range(3):
                    ps = p4p.tile([128, 512], F32, tag="p4")
                    nc.tensor.matmul(ps[:], lhsT=s4yr[c][:], rhs=yu[i][:],
                                     start=True, stop=False)
                    nc.tensor.matmul(ps[:], lhsT=s4cr[c][:], rhs=cu[i][:],
                                     start=False, stop=True)
                    ot = osb.tile([128, 512], F32, tag="ot")
                    nc.vector.tensor_scalar(out=ot[:], in0=ps[:], scalar1=1.0,
                                            scalar2=0.0, op0=AL.min, op1=AL.max)
                    nc.sync.dma_start(out[b, c, 128 * i:128 * (i + 1), :], ot[:])

    split_multi_waits(nc)
    return nc


# ---------------------------------------------------------------------------
# host entry point
# ---------------------------------------------------------------------------
_CACHE = {}


def _get_nc():
    if "nc" not in _CACHE:
        nc = bass.Bass("TRN2", target_bir_lowering=False, debug=False,
                       num_devices=N_CORES)
        build(nc)
        _CACHE["nc"] = nc
    return _CACHE["nc"]


def kernel(x: np.ndarray, source_image: np.ndarray, trace: bool = False,
           **run_kwargs):
    x = np.ascontiguousarray(np.asarray(x, dtype=np.float32))
    assert x.shape == (16, 3, 512, 512)
    nc = _get_nc()
    in_maps = [{"x": x[core * B_PER_CORE:(core + 1) * B_PER_CORE]}
               for core in range(N_CORES)]
    res = run_bass_kernel_spmd(nc, in_maps, core_ids=list(range(N_CORES)),
                               trace=trace, **run_kwargs)
    recovered = np.concatenate([res.results[c]["out"] for c in range(N_CORES)],
                               axis=0)
    _CACHE["last_results"] = res
    return recovered, source_image


# revision 21
# speedup vs baseline: 1.3328x; 1.0988x over previous
# Trainium2 Bass kernel for DiffJPEG (nn_DiffJPEG_40587440947390).
#
# Pipeline per image: RGB->YCbCr + 4:2:0 subsample + per-8x8-block DCT +
# quantize + differentiable round + dequantize + IDCT + upsample + YCbCr->RGB
# + clip. Everything linear is folded into PE matmul stationaries; the only
# elementwise work is quantize/round/cube/dequantize and the final clip.
# All +-128 shifts cancel exactly (integer shifts commute with round).
#
# Sharding: pure data parallel, 2 images per core across 8 cores.
import os
from contextlib import ExitStack

import numpy as np

import bass_rust
import concourse.bass as bass
import concourse.tile as tile
from concourse import mybir
from concourse.bass_utils import run_bass_kernel_spmd
from concourse.masks import make_identity

F32 = mybir.dt.float32
N_CORES = 8
B_PER_CORE = 2

# fwd matmuls feed diff_round (precision-sensitive); inverse side does not.
FWD_F32R = os.environ.get("K_FWD_F32R", "0") == "1"
T2_F32R = os.environ.get("K_T2_F32R", "1") == "1"

# ---------------------------------------------------------------------------
# constants (mirror reference.py exactly)
# ---------------------------------------------------------------------------
QUALITY = 80
FACTOR = (200.0 - 2.0 * QUALITY) / 100.0

Y_TABLE = np.array([
    [16, 11, 10, 16, 24, 40, 51, 61],
    [12, 12, 14, 19, 26, 58, 60, 55],
    [14, 13, 16, 24, 40, 57, 69, 56],
    [14, 17, 22, 29, 51, 87, 80, 62],
    [18, 22, 37, 56, 68, 109, 103, 77],
    [24, 35, 55, 64, 81, 104, 113, 92],
    [49, 64, 78, 87, 103, 121, 120, 101],
    [72, 92, 95, 98, 112, 100, 103, 99]], dtype=np.float32).T

C_TABLE = np.full((8, 8), 99.0, dtype=np.float32)
C_TABLE[:4, :4] = np.array([
    [17, 18, 24, 47],
    [18, 21, 26, 66],
    [24, 26, 56, 99],
    [47, 66, 99, 99]], dtype=np.float32).T

_k = np.arange(8, dtype=np.float32)
_COS = np.cos((2.0 * _k[:, None] + 1.0) * _k[None, :] * np.pi / 16.0).astype(np.float32)
_ALPHA = np.array([1.0 / np.sqrt(2.0)] + [1.0] * 7, dtype=np.float32)

RGB2YCC = np.array([[0.299, 0.587, 0.114],
                    [-0.168736, -0.331264, 0.5],
                    [0.5, -0.418688, -0.081312]], dtype=np.float32)
YCC2RGB = np.array([[1.0, 0.0, 1.402],
                    [1.0, -0.344136, -0.714136],
                    [1.0, 1.772, 0.0]], dtype=np.float32)

MAGIC = float(np.float32(1.5 * 2.0 ** 23))

G8 = (_COS * (_ALPHA[None, :] * 0.5)).astype(np.float32)     # [x, u]
H8 = (_COS.T * (_ALPHA[:, None] * 0.5)).astype(np.float32)   # [freq, pixel]


def _blockdiag(M, n):
    b0, b1 = M.shape
    out = np.zeros((b0 * n, b1 * n), dtype=np.float32)
    for i in range(n):
        out[i * b0:(i + 1) * b0, i * b1:(i + 1) * b1] = M
    return out


def _pool_mat(nrows):
    P = np.zeros((nrows, nrows // 2), dtype=np.float32)
    for h in range(nrows):
        P[h, h // 2] = 1.0
    return P


def _up_mat(nsmall):
    U = np.zeros((nsmall, 2 * nsmall), dtype=np.float32)
    for w in range(2 * nsmall):
        U[w // 2, w] = 1.0
    return U


def build_mats():
    m = {}
    BD16G = _blockdiag(G8, 16)
    BD16H = _blockdiag(H8, 16)
    BD8G = _blockdiag(G8, 8)
    BD8H = _blockdiag(H8, 8)
    P128 = _pool_mat(128)
    U64 = _up_mat(64)

    m['S1Y'] = [(np.float32(255.0 * RGB2YCC[0, c]) * BD16G).astype(np.float32)
                for c in range(3)]
    E = (P128 @ BD8G).astype(np.float32)
    m['S1C'] = [np.concatenate([
        np.float32(255.0 * 0.25 * RGB2YCC[1, c]) * E,
        np.float32(255.0 * 0.25 * RGB2YCC[2, c]) * E], axis=1).astype(np.float32)
        for c in range(3)]
    m['S2'] = BD16G
    m['S3'] = BD16H
    s3c = []
    for q in range(2):
        M2 = np.zeros((128, 128), dtype=np.float32)
        M2[64 * q:64 * q + 64, :] = BD8H @ U64
        s3c.append(M2.astype(np.float32))
    m['S3C'] = s3c
    m['S4Y'] = [(np.float32(YCC2RGB[c, 0] / 255.0) * BD16H).astype(np.float32)
                for c in range(3)]
    m['S4C'] = [np.concatenate([
        np.float32(YCC2RGB[c, 1] / 255.0) * (BD8H @ U64),
        np.float32(YCC2RGB[c, 2] / 255.0) * (BD8H @ U64)], axis=0).astype(np.float32)
        for c in range(3)]

    p = np.arange(128)[:, None]
    f = np.arange(512)[None, :]
    m['RY'] = (1.0 / (Y_TABLE[f % 8, p % 8] * FACTOR)).astype(np.float32)
    m['RYinv'] = (Y_TABLE[f % 8, p % 8] * FACTOR).astype(np.float32)
    m['RC'] = (1.0 / (C_TABLE[f % 8, p % 8] * FACTOR)).astype(np.float32)
    m['RCinv'] = (C_TABLE[f % 8, p % 8] * FACTOR).astype(np.float32)
    return m


def split_multi_waits(nc):
    """The walrus build in this container accepts at most one sync wait per
    instruction; Tile attaches several. Split the extras onto standalone
    NoOp carriers immediately before each instruction (same engine)."""
    cnt = 0
    for f in nc.m.functions:
        for bb in f.blocks:
            old = list(bb.instructions)
            new = []
            changed = False
            for inst in old:
                si = inst.sync_info
                if si is not None and len(si.on_wait) > 1:
                    waits = list(si.on_wait)
                    for w in waits[:-1]:
                        cnt += 1
                        nop = bass_rust.InstNoOp(name=f"wnop-{cnt}", ins=[],
                                                 outs=[])
                        nop.engine = inst.engine
                        nop.sync_info = mybir.SyncInfo(on_wait=[w],
                                                       on_update=[])
                        new.append(nop)
                    inst.sync_info = mybir.SyncInfo(
                        on_wait=[waits[-1]], on_update=list(si.on_update))
                    changed = True
                new.append(inst)
            if changed:
                bb.instructions = new
    return cnt


# ---------------------------------------------------------------------------
# kernel builder
# ---------------------------------------------------------------------------
def build(nc: bass.Bass):
    AL = mybir.AluOpType
    m = build_mats()

    x = nc.dram_tensor("x", [B_PER_CORE, 3, 512, 512], F32,
                       kind="ExternalInput").ap()
    out = nc.dram_tensor("out", [B_PER_CORE, 3, 512, 512], F32,
                         kind="ExternalOutput").ap()

    # constants baked into the NEFF
    s1y_d = nc.inline_tensor(np.stack(m['S1Y']), "c_s1y").ap()
    s1c_d = nc.inline_tensor(np.stack(m['S1C']), "c_s1c").ap()
    s2_d = nc.inline_tensor(m['S2'], "c_s2").ap()
    s3_d = nc.inline_tensor(m['S3'], "c_s3").ap()
    s3c_d = nc.inline_tensor(np.stack(m['S3C']), "c_s3c").ap()
    s4y_d = nc.inline_tensor(np.stack(m['S4Y']), "c_s4y").ap()
    s4c_d = nc.inline_tensor(np.stack(m['S4C']), "c_s4c").ap()
    r_d = nc.inline_tensor(np.stack([m['RY'], m['RYinv'], m['RC'], m['RCinv']]),
                           "c_r").ap()

    with tile.TileContext(nc) as tc, ExitStack() as ctx:
        cpool = ctx.enter_context(tc.tile_pool(name="consts", bufs=1))

        def cload(dram_ap, shape, tag):
            t = cpool.tile(shape, F32, tag=tag)
            nc.sync.dma_start(t[:], dram_ap)
            return t

        s1y = [cload(s1y_d[c], [128, 128], f"s1y{c}") for c in range(3)]
        s1c = [cload(s1c_d[c], [128, 128], f"s1c{c}") for c in range(3)]
        s2 = cload(s2_d, [128, 128], "s2")
        s3 = cload(s3_d, [128, 128], "s3")
        s3c = [cload(s3c_d[q], [128, 128], f"s3c{q}") for q in range(2)]
        s4y = [cload(s4y_d[c], [128, 128], f"s4y{c}") for c in range(3)]
        s4c = [cload(s4c_d[c], [128, 128], f"s4c{c}") for c in range(3)]
        rt = [cload(r_d[j], [128, 512], f"r{j}") for j in range(4)]
        RY, RYI, RC, RCI = rt

        ident = cpool.tile([128, 128], F32, tag="ident")
        make_identity(nc, ident[:])

        # float32r copies of inverse-side constants (PE requires fp32r
        # matmul inputs to be produced as rounded float32r)
        R32 = mybir.dt.float32r

        def to_r32(src_t, tag):
            t = cpool.tile([128, 128], R32, tag=tag)
            nc.scalar.copy(t[:], src_t[:])
            return t

        s3r = to_r32(s3, "s3r")
        s3cr = [to_r32(s3c[q], f"s3cr{q}") for q in range(2)]
        s4yr = [to_r32(s4y[c], f"s4yr{c}") for c in range(3)]
        s4cr = [to_r32(s4c[c], f"s4cr{c}") for c in range(3)]
        identr = to_r32(ident, "identr")

        # SBUF pools
        xp = ctx.enter_context(tc.tile_pool(name="x", bufs=16))
        wp = ctx.enter_context(tc.tile_pool(name="wpool", bufs=8))
        p1e = ctx.enter_context(tc.tile_pool(name="p1evac", bufs=6))
        t1e = ctx.enter_context(tc.tile_pool(name="t1evac", bufs=7))
        vch = ctx.enter_context(tc.tile_pool(name="vchain", bufs=4))
        yqp = ctx.enter_context(tc.tile_pool(name="yq", bufs=8))
        p3e = ctx.enter_context(tc.tile_pool(name="p3evac", bufs=6))
        t2e = ctx.enter_context(tc.tile_pool(name="t2evac", bufs=10))
        osb = ctx.enter_context(tc.tile_pool(name="outsb", bufs=6))

        # PSUM pools (8 banks total)
        p1p = ctx.enter_context(tc.tile_pool(name="p1p", bufs=2, space="PSUM"))
        p2p = ctx.enter_context(tc.tile_pool(name="p2p", bufs=1, space="PSUM"))
        tpp = ctx.enter_context(tc.tile_pool(name="tpp", bufs=1, space="PSUM"))
        p3p = ctx.enter_context(tc.tile_pool(name="p3p", bufs=1, space="PSUM"))
        p4p = ctx.enter_context(tc.tile_pool(name="p4p", bufs=2, space="PSUM"))

        def fcast(ap):
            return ap.bitcast(R32) if FWD_F32R else ap

        for b in range(B_PER_CORE):
            # ---- load x tiles ----
            xt = [[None] * 4 for _ in range(3)]
            for c in range(3):
                for i in range(4):
                    t = xp.tile([128, 512], F32, tag="x")
                    nc.sync.dma_start(t[:], x[b, c, 128 * i:128 * (i + 1), :])
                    xt[c][i] = t

            # ---- horizontal (w) pool for chroma path (gpsimd) ----
            pw = [[None] * 4 for _ in range(3)]
            for c in range(3):
                for i in range(4):
                    t = wp.tile([128, 256], F32, tag="pw")
                    xr = xt[c][i][:].rearrange("p (w t) -> p w t", t=2)
                    nc.gpsimd.tensor_tensor(
                        out=t[:], in0=xr[:, :, 0], in1=xr[:, :, 1], op=AL.add)
                    pw[c][i] = t

            # ---- pass-1 Y: [u, w] ----
            yt1 = []
            for i in range(4):
                ps = p1p.tile([128, 512], F32, tag="p1")
                for c in range(3):
                    nc.tensor.matmul(ps[:], lhsT=fcast(s1y[c][:]),
                                     rhs=fcast(xt[c][i][:]),
                                     start=(c == 0), stop=(c == 2))
                ev = p1e.tile([128, 512], F32, tag="yt1")
                nc.scalar.copy(ev[:], ps[:])
                yt1.append(ev)

            # ---- pass-1 chroma: [cb_u2|cr_u2, w2] ----
            ct1 = []
            for i in range(4):
                ps = p1p.tile([128, 256], F32, tag="p1")
                for c in range(3):
                    nc.tensor.matmul(ps[:], lhsT=fcast(s1c[c][:]),
                                     rhs=fcast(pw[c][i][:]),
                                     start=(c == 0), stop=(c == 2))
                ev = p1e.tile([128, 256], F32, tag="ct1")
                nc.scalar.copy(ev[:], ps[:])
                ct1.append(ev)

            # ---- T1 (PE transposes) ----
            yt1T = []
            for t_ in range(4):
                ps = tpp.tile([128, 512], F32, tag="tp")
                for i in range(4):
                    nc.tensor.transpose(
                        ps[:, 128 * i:128 * (i + 1)],
                        yt1[i][:, 128 * t_:128 * (t_ + 1)], ident[:])
                ev = t1e.tile([128, 512], F32, tag="t1ev")
                nc.vector.tensor_copy(out=ev[:], in_=ps[:])
                yt1T.append(ev)
            ct1T = []
            for k in range(2):
                ps = tpp.tile([128, 512], F32, tag="tp")
                for i in range(4):
                    nc.tensor.transpose(
                        ps[:, 128 * i:128 * (i + 1)],
                        ct1[i][:, 128 * k:128 * (k + 1)], ident[:])
                ev = t1e.tile([128, 512], F32, tag="t1ev")
                nc.scalar.copy(ev[:], ps[:])
                ct1T.append(ev)

            # ---- pass-2 + quant + diff_round + dequant ----
            def round_chain(zps, R, Rinv):
                v = vch.tile([128, 512], F32, tag="v")
                nc.vector.tensor_tensor(out=v[:], in0=zps[:], in1=R[:], op=AL.mult)
                r = vch.tile([128, 512], F32, tag="r")
                nc.vector.tensor_scalar(out=r[:], in0=v[:], scalar1=MAGIC,
                                        scalar2=MAGIC, op0=AL.add, op1=AL.subtract)
                d = vch.tile([128, 512], mybir.dt.bfloat16, tag="d")
                nc.gpsimd.tensor_tensor(out=d[:], in0=v[:], in1=r[:], op=AL.subtract)
                p2 = vch.tile([128, 512], mybir.dt.bfloat16, tag="p2")
                nc.scalar.square(p2[:], d[:])
                c3 = vch.tile([128, 512], mybir.dt.bfloat16, tag="c3")
                nc.vector.tensor_tensor(out=c3[:], in0=d[:], in1=p2[:], op=AL.mult)
                yq_q = vch.tile([128, 512], F32, tag="yqq")
                nc.vector.tensor_tensor(out=yq_q[:], in0=r[:], in1=c3[:], op=AL.add)
                yq = yqp.tile([128, 512], R32, tag="yq")
                nc.gpsimd.tensor_tensor(out=yq[:], in0=yq_q[:], in1=Rinv[:],
                                        op=AL.mult)
                return yq

            YQ = []
            for t_ in range(4):
                ps = p2p.tile([128, 512], F32, tag="p2")
                nc.tensor.matmul(ps[:], lhsT=fcast(s2[:]),
                                 rhs=fcast(yt1T[t_][:]),
                                 start=True, stop=True)
                YQ.append(round_chain(ps, RY, RYI))
            CQ = []
            for k in range(2):
                ps = p2p.tile([128, 512], F32, tag="p2")
                nc.tensor.matmul(ps[:], lhsT=fcast(s2[:]),
                                 rhs=fcast(ct1T[k][:]),
                                 start=True, stop=True)
                CQ.append(round_chain(ps, RC, RCI))

            # ---- pass-3 (float32r: post-round side tolerates reduced precision) ----
            py = []
            for t_ in range(4):
                ps = p3p.tile([128, 512], F32, tag="p3")
                nc.tensor.matmul(ps[:], lhsT=s3r[:], rhs=YQ[t_][:],
                                 start=True, stop=True)
                ev = p3e.tile([128, 512], R32, tag="p3ev")
                nc.scalar.copy(ev[:], ps[:])
                py.append(ev)
            pc = []
            for t_ in range(4):
                ps = p3p.tile([128, 512], F32, tag="p3")
                nc.tensor.matmul(ps[:], lhsT=s3cr[t_ % 2][:],
                                 rhs=CQ[t_ // 2][:],
                                 start=True, stop=True)
                ev = p3e.tile([128, 512], R32, tag="p3ev")
                nc.scalar.copy(ev[:], ps[:])
                pc.append(ev)

            # ---- T2 (fp32r transposes) ----
            yu, cu = [], []
            for i in range(4):
                ps = tpp.tile([128, 512], R32, tag="tpr")
                for t_ in range(4):
                    nc.tensor.transpose(
                        ps[:, 128 * t_:128 * (t_ + 1)],
                        py[t_][:, 128 * i:128 * (i + 1)], identr[:])
                ev = t2e.tile([128, 512], R32, tag="t2ev")
                nc.vector.tensor_copy(out=ev[:], in_=ps[:])
                yu.append(ev)
            for i in range(4):
                ps = tpp.tile([128, 512], R32, tag="tpr")
                for t_ in range(4):
                    nc.tensor.transpose(
                        ps[:, 128 * t_:128 * (t_ + 1)],
                        pc[t_][:, 128 * i:128 * (i + 1)], identr[:])
                ev = t2e.tile([128, 512], R32, tag="t2ev")
                nc.scalar.copy(ev[:], ps[:])
                cu.append(ev)

            # ---- pass-4 (+color) + clip + store ----
            for i in range(4):
                for c in range(3):
                    ps = p4p.tile([128, 512], F32, tag="p4")
                    nc.tensor.matmul(ps[:], lhsT=s4yr[c][:], rhs=yu[i][:],
                                     start=True, stop=False)
                    nc.tensor.matmul(ps[:], lhsT=s4cr[c][:], rhs=cu[i][:],
                                     start=False, stop=True)
                    ot = osb.tile([128, 512], F32, tag="ot")
                    nc.vector.tensor_scalar(out=ot[:], in0=ps[:], scalar1=1.0,
                                            scalar2=0.0, op0=AL.min, op1=AL.max)
                    nc.sync.dma_start(out[b, c, 128 * i:128 * (i + 1), :], ot[:])

    split_multi_waits(nc)
    return nc


# ---------------------------------------------------------------------------
# host entry point
# ---------------------------------------------------------------------------
_CACHE = {}


def _get_nc():
    if "nc" not in _CACHE:
        nc = bass.Bass("TRN2", target_bir_lowering=False, debug=False,
                       num_devices=N_CORES)
        build(nc)
        _CACHE["nc"] = nc
    return _CACHE["nc"]


def kernel(x: np.ndarray, source_image: np.ndarray, trace: bool = False,
           **run_kwargs):
    x = np.ascontiguousarray(np.asarray(x, dtype=np.float32))
    assert x.shape == (16, 3, 512, 512)
    nc = _get_nc()
    in_maps = [{"x": x[core * B_PER_CORE:(core + 1) * B_PER_CORE]}
               for core in range(N_CORES)]
    res = run_bass_kernel_spmd(nc, in_maps, core_ids=list(range(N_CORES)),
                               trace=trace, **run_kwargs)
    recovered = np.concatenate([res.results[c]["out"] for c in range(N_CORES)],
                               axis=0)
    _CACHE["last_results"] = res
    return recovered, source_image
